# revision 1
# baseline (speedup 1.0000x reference)
"""Trainium2 Bass kernel for a 2-layer GAT (PyG GATConv-style) over a
100k-node / 1.6M-edge random graph, distributed over 8 NeuronCores.

Strategy (dst-sharded graph parallel):
  - Nodes are split into 8 shards of 12544 (98 tiles of 128); edges are
    assigned to the core that owns their destination node. Per-core node->tile
    assignment is balanced by degree so every tile needs the same number of
    128-edge chunks.
  - Launch A: h1 = x @ [W1 | W1@att_src1 | W1@att_dst1] per node shard
    (TensorE matmul, f16 operands / f32 accumulate), producing the layer-1
    feature table (f16) and per-node attention scalars (f32). Node order is
    permuted by a two-level degree-balancing so every tile needs exactly
    ceil(max_tile_edges/128) = the pigeonhole-minimum number of chunks.
  - Host: concatenates the table (replicated per core), expands per-edge
    attention sums S1[e] = asrc1[src] + adst1[dst] into partition-major strips,
    and builds gather index / dst-slot strips.
  - Launch B: per dst tile, stream 128-edge chunks: indirect-DMA gather of
    source rows, ex = exp(leaky_relu(S1)) on ACT, one-hot(dst-slot) via
    is_equal vs an iota tile, and a single PSUM-accumulated TensorE matmul
    computing numerator and denominator together ([msg | ex] columns). The
    softmax max-subtraction is dropped: attention logits here are O(1) so
    exp() cannot overflow, and softmax is shift-invariant. Tile tail: divide,
    +b1, relu, transpose, @[W2 | W2@att_src2 | W2@att_dst2] -> rec2 table.
  - Launch C: same edge pass for layer 2 (40 features, 1 head) -> output.

The kernel is self-contained: it hardcodes shapes/sharding and only imports
the concourse (Bass) stack.
"""

import sys

for _p in ("/opt/trn_rl_repo", "/root/.axon_site/_ro/trn_rl_repo"):
    if _p not in sys.path:
        sys.path.insert(0, _p)

import numpy as np

import concourse.bass as bass
import concourse.bacc as bacc
import concourse.tile as tile
from concourse import mybir
from concourse import bass_utils

P = 128
N = 100000
NCORES = 8
SHARD = 12544            # nodes per core (incl. pad nodes on core 7)
NPAD = SHARD * NCORES    # 100352
TILES = SHARD // P       # 98
NFEAT, NHID, NCLASS, HEADS = 256, 16, 40, 8
F1 = HEADS * NHID        # 128
NEG = 0.2

F32 = mybir.dt.float32
F16 = mybir.dt.float16
I32 = mybir.dt.int32

GATHER_BUFS = 6


# ----------------------------------------------------------------- launch A
def build_launch_a(repeat=1, tiles=TILES, shard=SHARD):
    """h1ext = x_shard @ [W1 | W1as | W1ad] -> rec1 [shard,128] f16 + aux1 f32.
    DMAs grouped over GRP tiles to amortize HWDGE sequencer overhead."""
    nc = bacc.Bacc("TRN2", target_bir_lowering=False, debug=False,
                   enable_asserts=False, num_devices=NCORES)
    xT = nc.dram_tensor("xT", [NFEAT, shard], F16, kind="ExternalInput")
    w1x = nc.dram_tensor("w1x", [NFEAT, F1 + 2 * HEADS], F16, kind="ExternalInput")
    rec1 = nc.dram_tensor("rec1", [shard, F1], F16, kind="ExternalOutput")
    aux1 = nc.dram_tensor("aux1", [shard, 2 * HEADS], F32, kind="ExternalOutput")
    NC = F1 + 2 * HEADS  # 144
    GRP = 7 if tiles % 7 == 0 else 1
    NG = tiles // GRP
    GW = GRP * P
    with tile.TileContext(nc) as tc:
        with tc.tile_pool(name="w", bufs=1) as wp, \
             tc.tile_pool(name="x", bufs=3) as xp, \
             tc.tile_pool(name="o", bufs=3) as op, \
             tc.tile_pool(name="ps", bufs=3, space="PSUM") as pp:
            wt0 = wp.tile([P, NC], F16, tag="w0")
            wt1 = wp.tile([P, NC], F16, tag="w1")
            nc.sync.dma_start(out=wt0[:], in_=w1x[0:P, :])
            nc.sync.dma_start(out=wt1[:], in_=w1x[P:2 * P, :])
            rloop = tc.For_i(0, repeat, 1) if repeat > 1 else None
            if rloop is not None:
                rloop.__enter__()
            for g in range(NG):
                xt0 = xp.tile([P, GW], F16, tag="x0")
                xt1 = xp.tile([P, GW], F16, tag="x1")
                nc.sync.dma_start(out=xt0[:], in_=xT[0:P, g * GW:(g + 1) * GW])
                nc.sync.dma_start(out=xt1[:], in_=xT[P:2 * P, g * GW:(g + 1) * GW])
                ot = op.tile([P, GRP * F1], F16, tag="o")
                at = op.tile([P, GRP * 2 * HEADS], F32, tag="a")
                for j in range(GRP):
                    ps = pp.tile([P, NC], F32, tag="ps")
                    nc.tensor.matmul(ps[:], lhsT=xt0[:, j * P:(j + 1) * P],
                                     rhs=wt0[:], start=True, stop=False)
                    nc.tensor.matmul(ps[:], lhsT=xt1[:, j * P:(j + 1) * P],
                                     rhs=wt1[:], start=False, stop=True)
                    nc.vector.tensor_copy(out=ot[:, j * F1:(j + 1) * F1],
                                          in_=ps[:, 0:F1])
                    nc.vector.tensor_copy(
                        out=at[:, j * 2 * HEADS:(j + 1) * 2 * HEADS],
                        in_=ps[:, F1:NC])
                nc.sync.dma_start(
                    out=rec1[g * GW:(g + 1) * GW, :].rearrange(
                        "(j p) f -> p j f", p=P),
                    in_=ot[:].rearrange("p (j f) -> p j f", f=F1))
                nc.sync.dma_start(
                    out=aux1[g * GW:(g + 1) * GW, :].rearrange(
                        "(j p) f -> p j f", p=P),
                    in_=at[:].rearrange("p (j f) -> p j f", f=2 * HEADS))
            if rloop is not None:
                rloop.__exit__(None, None, None)
    nc.compile()
    return nc


# ------------------------------------------------------------- edge-pass core
def _iota_f16(nc, tc, pool):
    """[128,128] f16 tile with row content 0..127 (same every partition)."""
    it_i = pool.tile([P, P], I32, tag="iota_i")
    nc.gpsimd.iota(it_i[:], pattern=[[1, P]], base=0, channel_multiplier=0)
    it_f = pool.tile([P, P], F16, tag="iota_f")
    nc.vector.tensor_copy(out=it_f[:], in_=it_i[:])
    return it_f


def build_launch_b(C, repeat=1, tiles=TILES, shard=SHARD, npad=NPAD):
    """Layer-1 edge pass + layer-2 node transform.

    Inputs (per core):
      rec1    [npad, F1] f16   : full layer-1 feature table (replicated)
      selftab [shard, F1] f16  : own shard's rows (for the self-loop term)
      gidx    [P, tiles*C] i32 : gather row index per edge slot
      dstl    [P, tiles*C] f16 : dst slot within tile (-1 for pad slots)
      s1      [P, tiles*C*H]   : asrc1[src]+adst1[dst] per edge slot (f32)
      s1s     [P, tiles*H] f32 : same for the self loops, per (slot, tile)
      w2x     [P, 42] f16      : [W2 | W2 att_src2 | W2 att_dst2]
      b1t     [P, F1] f32      : b1 broadcast to 128 rows
    Output: rec2 [shard, 64] f16 (h2 | asrc2 | adst2 | zero-pad)
    """
    H = HEADS
    NCHUNK = tiles * C
    RHS = F1 + H  # 136
    nc = bacc.Bacc("TRN2", target_bir_lowering=False, debug=False,
                   enable_asserts=False, num_devices=NCORES)
    rec1 = nc.dram_tensor("rec1", [npad, F1], F16, kind="ExternalInput")
    selftab = nc.dram_tensor("selftab", [shard, F1], F16, kind="ExternalInput")
    gidx = nc.dram_tensor("gidx", [P, NCHUNK], I32, kind="ExternalInput")
    dstl = nc.dram_tensor("dstl", [P, NCHUNK], F16, kind="ExternalInput")
    s1 = nc.dram_tensor("s1", [P, NCHUNK * H], F32, kind="ExternalInput")
    s1s = nc.dram_tensor("s1s", [P, tiles * H], F32, kind="ExternalInput")
    w2x = nc.dram_tensor("w2x", [P, NCLASS + 2], F16, kind="ExternalInput")
    b1t = nc.dram_tensor("b1t", [P, F1], F32, kind="ExternalInput")
    rec2 = nc.dram_tensor("rec2", [shard, 64], F16, kind="ExternalOutput")

    with tile.TileContext(nc) as tc:
        with tc.tile_pool(name="static", bufs=1) as sp, \
             tc.tile_pool(name="g", bufs=GATHER_BUFS) as gp, \
             tc.tile_pool(name="rhs", bufs=3) as rp, \
             tc.tile_pool(name="oh", bufs=3) as ohp, \
             tc.tile_pool(name="sm", bufs=3) as smp, \
             tc.tile_pool(name="tl", bufs=2) as tlp, \
             tc.tile_pool(name="ps", bufs=3, space="PSUM") as pp, \
             tc.tile_pool(name="ps2", bufs=2, space="PSUM") as pp2:
            iota = _iota_f16(nc, tc, sp)
            ident = sp.tile([P, P], F32, tag="ident")
            from concourse.masks import make_identity
            make_identity(nc, ident[:])
            idx_t = sp.tile([P, NCHUNK], I32, tag="idx")
            nc.sync.dma_start(out=idx_t[:], in_=gidx[:, :])
            dst_t = sp.tile([P, NCHUNK], F16, tag="dst")
            nc.sync.dma_start(out=dst_t[:], in_=dstl[:, :])
            s1_t = sp.tile([P, NCHUNK * H], F32, tag="s1")
            nc.sync.dma_start(out=s1_t[:], in_=s1[:, :])
            w2t = sp.tile([P, NCLASS + 2], F16, tag="w2")
            nc.sync.dma_start(out=w2t[:], in_=w2x[:, :])
            b1s = sp.tile([P, F1], F32, tag="b1")
            nc.sync.dma_start(out=b1s[:], in_=b1t[:, :])
            s1s_t = sp.tile([P, tiles * H], F32, tag="s1s")
            nc.sync.dma_start(out=s1s_t[:], in_=s1s[:, :])

            rloop = tc.For_i(0, repeat, 1) if repeat > 1 else None
            if rloop is not None:
                rloop.__enter__()
            for t in range(tiles):
                ps = pp.tile([P, RHS], F32, tag="ps")
                selfr = gp.tile([P, F1], F16, tag="selfr")
                nc.sync.dma_start(out=selfr[:], in_=selftab[t * P:(t + 1) * P, :])
                # batched leaky-relu + exp for the whole tile (+ self cols)
                CW = C * H
                sm = smp.tile([P, CW + H], F32, tag="sm")
                es = smp.tile([P, CW + H], F32, tag="es")
                exs = smp.tile([P, CW + H], F32, tag="exs")
                nc.vector.tensor_scalar_mul(
                    out=sm[:, 0:CW], in0=s1_t[:, t * CW:(t + 1) * CW], scalar1=NEG)
                nc.vector.tensor_scalar_mul(
                    out=sm[:, CW:], in0=s1s_t[:, t * H:(t + 1) * H], scalar1=NEG)
                nc.vector.tensor_tensor(
                    out=es[:, 0:CW], in0=s1_t[:, t * CW:(t + 1) * CW],
                    in1=sm[:, 0:CW], op=mybir.AluOpType.max)
                nc.vector.tensor_tensor(
                    out=es[:, CW:], in0=s1s_t[:, t * H:(t + 1) * H],
                    in1=sm[:, CW:], op=mybir.AluOpType.max)
                nc.scalar.activation(out=exs[:], in_=es[:],
                                     func=mybir.ActivationFunctionType.Exp)
                for k in range(C):
                    col = t * C + k
                    g = gp.tile([P, F1], F16, tag="g")
                    nc.gpsimd.indirect_dma_start(
                        out=g[:], out_offset=None, in_=rec1[:, :],
                        in_offset=bass.IndirectOffsetOnAxis(
                            ap=idx_t[:, col:col + 1], axis=0),
                    )
                    rhs = rp.tile([P, RHS], F16, tag="rhs")
                    nc.vector.tensor_copy(out=rhs[:, F1:RHS],
                                          in_=exs[:, k * H:(k + 1) * H])
                    nc.vector.tensor_tensor(
                        out=rhs[:, 0:F1].rearrange("p (h c) -> p h c", h=H),
                        in0=g[:].rearrange("p (h c) -> p h c", h=H),
                        in1=exs[:, k * H:(k + 1) * H].to_broadcast([P, H, NHID]),
                        op=mybir.AluOpType.mult)
                    oh = ohp.tile([P, P], F16, tag="oh")
                    nc.vector.tensor_tensor(
                        out=oh[:], in0=dst_t[:, col:col + 1].to_broadcast([P, P]),
                        in1=iota[:], op=mybir.AluOpType.is_equal)
                    nc.tensor.matmul(ps[:], lhsT=oh[:], rhs=rhs[:],
                                     start=(k == 0), stop=(k == C - 1))
                # tile tail: self-loop term, divide, +b1, relu, transpose, @w2x
                den = tlp.tile([P, H], F32, tag="den")
                nc.vector.tensor_tensor(out=den[:], in0=ps[:, F1:RHS],
                                        in1=exs[:, CW:], op=mybir.AluOpType.add)
                num = tlp.tile([P, F1], F32, tag="num")
                nc.vector.tensor_tensor(
                    out=num[:].rearrange("p (h c) -> p h c", h=H),
                    in0=selfr[:].rearrange("p (h c) -> p h c", h=H),
                    in1=exs[:, CW:].to_broadcast([P, H, NHID]),
                    op=mybir.AluOpType.mult)
                nc.vector.tensor_tensor(out=num[:], in0=num[:], in1=ps[:, 0:F1],
                                        op=mybir.AluOpType.add)
                rcp = tlp.tile([P, H], F32, tag="rcp")
                nc.vector.reciprocal(out=rcp[:], in_=den[:])
                h1o = tlp.tile([P, F1], F32, tag="h1o")
                nc.vector.tensor_tensor(
                    out=h1o[:].rearrange("p (h c) -> p h c", h=H),
                    in0=num[:].rearrange("p (h c) -> p h c", h=H),
                    in1=rcp[:].to_broadcast([P, H, NHID]),
                    op=mybir.AluOpType.mult)
                nc.vector.tensor_tensor(out=h1o[:], in0=h1o[:], in1=b1s[:],
                                        op=mybir.AluOpType.add)
                h1r = tlp.tile([P, F1], F32, tag="h1r")
                nc.vector.tensor_scalar_max(out=h1r[:], in0=h1o[:], scalar1=0.0)
                psT = pp.tile([P, P], F32, tag="psT")
                nc.tensor.transpose(out=psT[:], in_=h1r[:], identity=ident[:])
                h1T = tlp.tile([P, P], F16, tag="h1T")
                nc.vector.tensor_copy(out=h1T[:], in_=psT[:])
                ps2 = pp2.tile([P, NCLASS + 2], F32, tag="ps2")
                nc.tensor.matmul(ps2[:], lhsT=h1T[:], rhs=w2t[:],
                                 start=True, stop=True)
                r2 = tlp.tile([P, NCLASS + 2], F16, tag="r2")
                nc.vector.tensor_copy(out=r2[:], in_=ps2[:])
                nc.sync.dma_start(out=rec2[t * P:(t + 1) * P, 0:NCLASS + 2],
                                  in_=r2[:])
            if rloop is not None:
                rloop.__exit__(None, None, None)
    nc.compile()
    return nc


def build_launch_c(C, repeat=1, tiles=TILES, shard=SHARD, npad=NPAD):
    """Layer-2 edge pass -> out [SHARD, 40] f32."""
    NCHUNK = tiles * C
    RHS = NCLASS + 1  # 41
    nc = bacc.Bacc("TRN2", target_bir_lowering=False, debug=False,
                   enable_asserts=False, num_devices=NCORES)
    rec2 = nc.dram_tensor("rec2", [npad, 64], F16, kind="ExternalInput")
    selftab = nc.dram_tensor("selftab", [shard, 64], F16, kind="ExternalInput")
    gidx = nc.dram_tensor("gidx", [P, NCHUNK], I32, kind="ExternalInput")
    dstl = nc.dram_tensor("dstl", [P, NCHUNK], F16, kind="ExternalInput")
    s2 = nc.dram_tensor("s2", [P, NCHUNK], F32, kind="ExternalInput")
    s2s = nc.dram_tensor("s2s", [P, tiles], F32, kind="ExternalInput")
    b2t = nc.dram_tensor("b2t", [P, NCLASS], F32, kind="ExternalInput")
    outd = nc.dram_tensor("out", [shard, NCLASS], F32, kind="ExternalOutput")

    with tile.TileContext(nc) as tc:
        with tc.tile_pool(name="static", bufs=1) as sp, \
             tc.tile_pool(name="g", bufs=GATHER_BUFS) as gp, \
             tc.tile_pool(name="rhs", bufs=3) as rp, \
             tc.tile_pool(name="oh", bufs=3) as ohp, \
             tc.tile_pool(name="sm", bufs=3) as smp, \
             tc.tile_pool(name="tl", bufs=2) as tlp, \
             tc.tile_pool(name="ps", bufs=3, space="PSUM") as pp:
            iota = _iota_f16(nc, tc, sp)
            idx_t = sp.tile([P, NCHUNK], I32, tag="idx")
            nc.sync.dma_start(out=idx_t[:], in_=gidx[:, :])
            dst_t = sp.tile([P, NCHUNK], F16, tag="dst")
            nc.sync.dma_start(out=dst_t[:], in_=dstl[:, :])
            s2_t = sp.tile([P, NCHUNK], F32, tag="s2")
            nc.sync.dma_start(out=s2_t[:], in_=s2[:, :])
            b2s = sp.tile([P, NCLASS], F32, tag="b2")
            nc.sync.dma_start(out=b2s[:], in_=b2t[:, :])
            s2s_t = sp.tile([P, tiles], F32, tag="s2s")
            nc.sync.dma_start(out=s2s_t[:], in_=s2s[:, :])

            rloop = tc.For_i(0, repeat, 1) if repeat > 1 else None
            if rloop is not None:
                rloop.__enter__()
            for t in range(tiles):
                ps = pp.tile([P, RHS], F32, tag="ps")
                selfr = gp.tile([P, 64], F16, tag="selfr")
                nc.sync.dma_start(out=selfr[:], in_=selftab[t * P:(t + 1) * P, :])
                sm = smp.tile([P, C + 1], F32, tag="sm")
                es = smp.tile([P, C + 1], F32, tag="es")
                exs = smp.tile([P, C + 1], F32, tag="exs")
                nc.vector.tensor_scalar_mul(
                    out=sm[:, 0:C], in0=s2_t[:, t * C:(t + 1) * C], scalar1=NEG)
                nc.vector.tensor_scalar_mul(
                    out=sm[:, C:], in0=s2s_t[:, t:t + 1], scalar1=NEG)
                nc.vector.tensor_tensor(
                    out=es[:, 0:C], in0=s2_t[:, t * C:(t + 1) * C],
                    in1=sm[:, 0:C], op=mybir.AluOpType.max)
                nc.vector.tensor_tensor(
                    out=es[:, C:], in0=s2s_t[:, t:t + 1], in1=sm[:, C:],
                    op=mybir.AluOpType.max)
                nc.scalar.activation(out=exs[:], in_=es[:],
                                     func=mybir.ActivationFunctionType.Exp)
                for k in range(C):
                    col = t * C + k
                    g = gp.tile([P, 64], F16, tag="g")
                    nc.gpsimd.indirect_dma_start(
                        out=g[:], out_offset=None, in_=rec2[:, :],
                        in_offset=bass.IndirectOffsetOnAxis(
                            ap=idx_t[:, col:col + 1], axis=0),
                    )
                    rhs = rp.tile([P, RHS], F16, tag="rhs")
                    nc.vector.tensor_copy(out=rhs[:, NCLASS:RHS],
                                          in_=exs[:, k:k + 1])
                    nc.vector.tensor_tensor(
                        out=rhs[:, 0:NCLASS],
                        in0=g[:, 0:NCLASS],
                        in1=exs[:, k:k + 1].to_broadcast([P, NCLASS]),
                        op=mybir.AluOpType.mult)
                    oh = ohp.tile([P, P], F16, tag="oh")
                    nc.vector.tensor_tensor(
                        out=oh[:], in0=dst_t[:, col:col + 1].to_broadcast([P, P]),
                        in1=iota[:], op=mybir.AluOpType.is_equal)
                    nc.tensor.matmul(ps[:], lhsT=oh[:], rhs=rhs[:],
                                     start=(k == 0), stop=(k == C - 1))
                den = tlp.tile([P, 1], F32, tag="den")
                nc.vector.tensor_tensor(out=den[:], in0=ps[:, NCLASS:RHS],
                                        in1=exs[:, C:], op=mybir.AluOpType.add)
                num = tlp.tile([P, NCLASS], F32, tag="num")
                nc.vector.tensor_scalar_mul(out=num[:], in0=selfr[:, 0:NCLASS],
                                            scalar1=exs[:, C:C + 1])
                nc.vector.tensor_tensor(out=num[:], in0=num[:], in1=ps[:, 0:NCLASS],
                                        op=mybir.AluOpType.add)
                rcp = tlp.tile([P, 1], F32, tag="rcp")
                nc.vector.reciprocal(out=rcp[:], in_=den[:])
                ot = tlp.tile([P, NCLASS], F32, tag="ot")
                nc.vector.tensor_scalar_mul(out=ot[:], in0=num[:],
                                            scalar1=rcp[:, 0:1])
                nc.vector.tensor_tensor(out=ot[:], in0=ot[:], in1=b2s[:],
                                        op=mybir.AluOpType.add)
                nc.sync.dma_start(out=outd[t * P:(t + 1) * P, :], in_=ot[:])
            if rloop is not None:
                rloop.__exit__(None, None, None)
    nc.compile()
    return nc


# ------------------------------------------------------------- host prep
def host_prep(edge_index):
    """Edge partitioning with two-level degree balancing (nodes->cores,
    nodes->tiles within core) so every tile needs the same number of
    128-edge chunks (the pigeonhole minimum). Self-loops are handled as a
    separate per-tile dense term."""
    import bisect

    src = np.asarray(edge_index[0], dtype=np.int64)
    dst = np.asarray(edge_index[1], dtype=np.int64)
    E = src.shape[0]

    deg = np.bincount(dst, minlength=NPAD)  # without self loops

    def balance(items_deg, nbins, bin_cap, max_iters):
        """Assign len(items_deg) items into nbins bins of exactly bin_cap,
        minimizing the max bin degree-sum. Returns bin id per item."""
        n = len(items_deg)
        order = np.argsort(-items_deg, kind="stable")
        rounds = np.arange(n) // nbins
        pos = np.arange(n) % nbins
        bin_of_rank = np.where(rounds % 2 == 0, pos, nbins - 1 - pos)
        bin_id = np.empty(n, dtype=np.int64)
        bin_id[order] = bin_of_rank
        sums = np.bincount(bin_id, weights=items_deg, minlength=nbins).astype(np.int64)
        members = [sorted(np.where(bin_id == b)[0], key=lambda i: items_deg[i])
                   for b in range(nbins)]
        keyf = lambda i: items_deg[i]
        for _ in range(max_iters):
            bmax = int(np.argmax(sums)); bmin = int(np.argmin(sums))
            gap = sums[bmax] - sums[bmin]
            if gap <= 1:
                break
            hi = members[bmax][-1]
            want = items_deg[hi] - (gap + 1) // 2
            degs_min = [items_deg[i] for i in members[bmin]]
            j = min(max(bisect.bisect_left(degs_min, want), 0),
                    len(members[bmin]) - 1)
            lo = members[bmin][j]
            delta = items_deg[hi] - items_deg[lo]
            if delta <= 0:
                break
            members[bmax].pop(); members[bmin].pop(j)
            bisect.insort(members[bmax], lo, key=keyf)
            bisect.insort(members[bmin], hi, key=keyf)
            sums[bmax] -= delta; sums[bmin] += delta
            bin_id[hi] = bmin; bin_id[lo] = bmax
        return bin_id

    # level 1: nodes -> cores
    node_core = balance(deg.astype(np.int64), NCORES, SHARD, 4000)

    # level 2: nodes -> (tile, slot) within each core
    perm_row = np.empty(NPAD, dtype=np.int64)   # node -> table row
    inv = np.empty((NCORES, SHARD), dtype=np.int64)  # (core,pos) -> node
    for c in range(NCORES):
        members_c = np.where(node_core == c)[0]
        tile_of = balance(deg[members_c].astype(np.int64), TILES, P, 3000)
        pos_in_tile = np.zeros(SHARD, dtype=np.int64)
        cnt = np.zeros(TILES, dtype=np.int64)
        for i in range(SHARD):
            t = tile_of[i]
            pos_in_tile[i] = cnt[t]; cnt[t] += 1
        pos = tile_of * P + pos_in_tile
        perm_row[members_c] = c * SHARD + pos
        inv[c, pos] = members_c

    # edge -> (core, tile, slot)
    e_core = node_core[dst]
    e_pos = perm_row[dst] % SHARD
    e_tile = e_pos // P
    e_slot = e_pos % P

    counts = np.zeros((NCORES, TILES), dtype=np.int64)
    np.add.at(counts, (e_core, e_tile), 1)
    C = int(np.ceil(counts.max() / P))

    order = np.lexsort((src, e_tile, e_core))
    o_src = src[order]; o_tile = e_tile[order]; o_slot = e_slot[order]
    o_core = e_core[order]

    NS = TILES * C * P
    g_src = np.zeros((NCORES, NS), dtype=np.int64)
    g_slot = np.full((NCORES, NS), -1, dtype=np.float16)
    g_dstid = np.zeros((NCORES, NS), dtype=np.int64)
    core_starts = np.searchsorted(o_core, np.arange(NCORES + 1))
    for c in range(NCORES):
        lo_i, hi_i = core_starts[c], core_starts[c + 1]
        ct = o_tile[lo_i:hi_i]; cs = o_src[lo_i:hi_i]; cl = o_slot[lo_i:hi_i]
        tstarts = np.searchsorted(ct, np.arange(TILES + 1))
        for t in range(TILES):
            a, b = tstarts[t], tstarts[t + 1]
            n = b - a
            base = t * C * P
            g_src[c, base:base + n] = cs[a:b]
            g_slot[c, base:base + n] = cl[a:b].astype(np.float16)
            g_dstid[c, base:base + n] = inv[c, (t * P + cl[a:b]).astype(np.int64)]

    def pm(arr_c, width=1):
        a = arr_c.reshape(TILES * C, P, width)
        a = np.transpose(a, (1, 0, 2)).reshape(P, TILES * C * width)
        return np.ascontiguousarray(a)

    return dict(C=C, g_src=g_src, g_slot=g_slot, g_dstid=g_dstid,
                perm_row=perm_row, inv=inv, pm=pm, E=E)


def _weights_ext(W1, att_src1, att_dst1, W2, att_src2, att_dst2):
    W1r = W1.reshape(NFEAT, HEADS, NHID)
    W1as = np.einsum('fhc,hc->fh', W1r, att_src1).astype(np.float32)
    W1ad = np.einsum('fhc,hc->fh', W1r, att_dst1).astype(np.float32)
    w1x = np.concatenate([W1, W1as, W1ad], axis=1).astype(np.float32)
    W2as = (W2 @ att_src2[0]).astype(np.float32)[:, None]
    W2ad = (W2 @ att_dst2[0]).astype(np.float32)[:, None]
    w2x = np.concatenate([W2, W2as, W2ad], axis=1).astype(np.float16)
    return w1x, w2x


def kernel(x, edge_index, W1, att_src1, att_dst1, b1, W2, att_src2, att_dst2,
           b2, _collect=None):
    x = np.asarray(x, dtype=np.float32)
    w1x, w2x = _weights_ext(
        np.asarray(W1, np.float32), np.asarray(att_src1, np.float32),
        np.asarray(att_dst1, np.float32), np.asarray(W2, np.float32),
        np.asarray(att_src2, np.float32), np.asarray(att_dst2, np.float32))
    b1 = np.asarray(b1, np.float32)
    b2 = np.asarray(b2, np.float32)

    prep = host_prep(edge_index)
    C = prep["C"]; pm = prep["pm"]

    perm_row = prep["perm_row"]

    # ---- launch A (inputs permuted so outputs are in tile order)
    xpad = np.zeros((NPAD, NFEAT), np.float32)
    xpad[:N] = x
    nc_a = build_launch_a()
    in_a = []
    for c in range(NCORES):
        xT = np.ascontiguousarray(xpad[prep["inv"][c]].T).astype(np.float16)
        in_a.append({"xT": xT, "w1x": w1x.astype(np.float16)})
    res_a = bass_utils.run_bass_kernel_spmd(nc_a, in_a, core_ids=list(range(NCORES)))
    if _collect is not None:
        _collect["A"] = (in_a, C)
    rec1 = np.concatenate([res_a.results[c]["rec1"] for c in range(NCORES)], 0)
    aux1 = np.concatenate([res_a.results[c]["aux1"] for c in range(NCORES)], 0)

    # ---- strips for B (tables are in permuted order; index via perm_row)
    src_rows = perm_row[prep["g_src"]].astype(np.int32)
    dst_rows = perm_row[prep["g_dstid"]].astype(np.int32)
    s1_e = aux1[src_rows, 0:HEADS] + aux1[dst_rows, HEADS:2 * HEADS]
    s1s = aux1[:, 0:HEADS] + aux1[:, HEADS:2 * HEADS]  # per node, self term
    nc_b = build_launch_b(C)
    b1t = np.broadcast_to(b1, (P, F1)).copy()
    in_b = []
    for c in range(NCORES):
        s1s_c = s1s[c * SHARD:(c + 1) * SHARD].reshape(TILES, P, HEADS)
        s1s_c = np.ascontiguousarray(
            np.transpose(s1s_c, (1, 0, 2)).reshape(P, TILES * HEADS))
        in_b.append({
            "rec1": rec1,
            "selftab": rec1[c * SHARD:(c + 1) * SHARD],
            "gidx": pm(src_rows[c]).astype(np.int32),
            "dstl": pm(prep["g_slot"][c].astype(np.float16)).astype(np.float16),
            "s1": pm(s1_e[c].astype(np.float32), HEADS).astype(np.float32),
            "s1s": s1s_c.astype(np.float32),
            "w2x": w2x, "b1t": b1t,
        })
    if _collect is not None:
        _collect["B"] = (in_b, C)
    res_b = bass_utils.run_bass_kernel_spmd(nc_b, in_b, core_ids=list(range(NCORES)))
    rec2 = np.concatenate([res_b.results[c]["rec2"] for c in range(NCORES)], 0)

    # ---- strips for C
    r2f = rec2.astype(np.float32)
    s2_e = r2f[src_rows, NCLASS] + r2f[dst_rows, NCLASS + 1]
    s2s = r2f[:, NCLASS] + r2f[:, NCLASS + 1]
    nc_c = build_launch_c(C)
    b2t = np.broadcast_to(b2, (P, NCLASS)).copy()
    in_c = []
    for c in range(NCORES):
        s2s_c = s2s[c * SHARD:(c + 1) * SHARD].reshape(TILES, P)
        s2s_c = np.ascontiguousarray(s2s_c.T)
        in_c.append({
            "rec2": rec2,
            "selftab": rec2[c * SHARD:(c + 1) * SHARD],
            "gidx": pm(src_rows[c]).astype(np.int32),
            "dstl": pm(prep["g_slot"][c].astype(np.float16)).astype(np.float16),
            "s2": pm(s2_e[c].astype(np.float32)).astype(np.float32),
            "s2s": s2s_c.astype(np.float32),
            "b2t": b2t,
        })
    if _collect is not None:
        _collect["C"] = (in_c, C)
    res_c = bass_utils.run_bass_kernel_spmd(nc_c, in_c, core_ids=list(range(NCORES)))
    out_perm = np.concatenate([res_c.results[c]["out"] for c in range(NCORES)], 0)

    # node n's output lives at permuted row perm_row[n]
    out = out_perm[perm_row]
    return np.ascontiguousarray(out[:N])

